# revision 1
# baseline (speedup 1.0000x reference)
"""Trainium2 Bass kernel for nn_ChunkedAttention (B=4, T=4096, D=1024, H=16, dh=64).

Sharding: 8 cores = 4 batches x 2 head-groups (8 heads each). Host sums the
two partial out-projections per batch.

All inputs bf16 (host passes x^T, so no PE transposes). Single software
pipeline: attention(qg=t) drains a fill queue holding proj(t+1) chains and
outproj(t-1) pieces into the PE's spare cycles, so the softmax (Act) engine
never waits on a dedicated projection phase and the PE never idles long
enough for the HAM clock gate to re-throttle it. Scores use bf16 row-tiled
pairs; AV(kt-1) is emitted after exp(kt) so scores(kt+1) overlaps the exp;
softmax reciprocals are batched [8,512] per query group; the last group
normalizes per head pair so the final out-projection isn't tail-serialized.
Host sums the two partial out-projections per batch in fp32.
"""

import os
import sys

import numpy as np

for _p in ("/opt/trn_rl_repo",):
    if _p not in sys.path and os.path.isdir(_p):
        sys.path.insert(0, _p)

import ml_dtypes

import concourse.bass as bass
import concourse.mybir as mybir
import concourse.tile as tile
from concourse.bacc import Bacc
from concourse.bass_utils import run_bass_kernel_spmd

F32 = mybir.dt.float32
BF16 = mybir.dt.bfloat16
EXP = mybir.ActivationFunctionType.Exp
MULT = mybir.AluOpType.mult

B, T, D = 4, 4096, 1024
HG = 512          # head-group width per core (8 heads x 64)
NH, DH = 8, 64    # heads per core, head dim
NPAIR = 4         # head pairs per core
QG = 512          # query-group width
NQG = T // QG     # 8
NKT = T // 128    # 32 k-tiles
NDC = D // 128    # 8 d_model chunks
SCALE = 1.0 / np.sqrt(DH)  # 0.125

BF = ml_dtypes.bfloat16


def build_nc():
    nc = Bacc()
    xT_d = nc.dram_tensor("xT", [D, T], BF16, kind="ExternalInput")
    wq_d = nc.dram_tensor("wq", [D, HG], BF16, kind="ExternalInput")
    wk_d = nc.dram_tensor("wk", [D, HG], BF16, kind="ExternalInput")
    wv_d = nc.dram_tensor("wv", [D, HG], BF16, kind="ExternalInput")
    wo_d = nc.dram_tensor("wo", [HG, D], BF16, kind="ExternalInput")
    tri_d = nc.dram_tensor("tri", [128, 128], BF16, kind="ExternalInput")
    y_d = nc.dram_tensor("y", [T, D], BF16, kind="ExternalOutput")

    with tile.TileContext(nc) as tc_:
        with (
            tc_.tile_pool(name="const", bufs=1) as pconst,
            tc_.tile_pool(name="pxt", bufs=2) as pxt,
            tc_.tile_pool(name="pq", bufs=2) as pq,
            tc_.tile_pool(name="pet", bufs=4) as pet,
            tc_.tile_pool(name="pnrm", bufs=2) as pnrm,
            tc_.tile_pool(name="prb", bufs=3) as prb,
            tc_.tile_pool(name="psS", bufs=2, space="PSUM") as psS,
            tc_.tile_pool(name="psAV", bufs=2, space="PSUM") as psAV,
            tc_.tile_pool(name="psM", bufs=2, space="PSUM") as psM,
        ):
            kt_sb = pconst.tile([128, NPAIR, T], BF16, tag="kt")
            v_sb = pconst.tile([128, NKT, NH, DH + 1], BF16, tag="v")
            tri_sb = pconst.tile([128, 128], BF16, tag="tri")
            wq_sb = pconst.tile([128, NDC, HG], BF16, tag="wq")
            wk_sb = pconst.tile([128, NDC, HG], BF16, tag="wk")
            wv_sb = pconst.tile([128, NDC, HG], BF16, tag="wv")
            wo_sb = pconst.tile([128, NPAIR, D], BF16, tag="wo")
            xT_r = xT_d.rearrange("(dc p) t -> p dc t", p=128)

            def load_xt_first(tcn):
                xt = pxt.tile([128, NDC, QG], BF16, tag="xt", name="xt")
                nc.sync.dma_start(xt[:], xT_r[:, :, tcn * QG : (tcn + 1) * QG])
                return xt

            # first compute input (x^T chunk 0 + Wq) before the rest of the
            # constants, so the first projection chain starts ASAP
            xt0 = load_xt_first(0)
            nc.sync.dma_start(wq_sb[:], wq_d.rearrange("(dc p) h -> p dc h", p=128))
            nc.sync.dma_start(wk_sb[:], wk_d.rearrange("(dc p) h -> p dc h", p=128))
            nc.sync.dma_start(wv_sb[:], wv_d.rearrange("(dc p) h -> p dc h", p=128))
            nc.sync.dma_start(tri_sb[:], tri_d[:])
            nc.sync.dma_start(wo_sb[:], wo_d.rearrange("(hp p) e -> p hp e", p=128))
            nc.gpsimd.memset(v_sb[:, :, :, DH : DH + 1], 1.0)

            def load_xt(tcn):
                xt = pxt.tile([128, NDC, QG], BF16, tag="xt", name="xt")
                nc.sync.dma_start(xt[:], xT_r[:, :, tcn * QG : (tcn + 1) * QG])
                return xt

            def q_chain(tcn, xt, qst, hp):
                pq_ps = psM.tile([128, QG], F32, tag="mm", name="pq_ps")
                for dc in range(NDC):
                    nc.tensor.matmul(
                        pq_ps[:],
                        wq_sb[:, dc, hp * 128 : (hp + 1) * 128],
                        xt[:, dc, :],
                        start=(dc == 0),
                        stop=(dc == NDC - 1),
                    )
                nc.vector.tensor_copy(qst[:, hp, :], pq_ps[:])

            def k_chain(tcn, xt, hp):
                pk_ps = psM.tile([128, QG], F32, tag="mm", name="pk_ps")
                for dc in range(NDC):
                    nc.tensor.matmul(
                        pk_ps[:],
                        wk_sb[:, dc, hp * 128 : (hp + 1) * 128],
                        xt[:, dc, :],
                        start=(dc == 0),
                        stop=(dc == NDC - 1),
                    )
                nc.vector.tensor_copy(
                    kt_sb[:, hp, tcn * QG : (tcn + 1) * QG], pk_ps[:]
                )

            def v_chain(tcn, xt, ts):
                pv_ps = psM.tile([128, QG], F32, tag="mm", name="pv_ps")
                for dc in range(NDC):
                    nc.tensor.matmul(
                        pv_ps[:],
                        xt[:, dc, ts * 128 : (ts + 1) * 128],
                        wv_sb[:, dc, :],
                        start=(dc == 0),
                        stop=(dc == NDC - 1),
                    )
                ktg = tcn * 4 + ts
                nc.vector.tensor_copy(
                    v_sb[:, ktg, :, 0:DH],
                    pv_ps.rearrange("p (h d) -> p h d", h=NH),
                )

            def proj_items(tcn, xt, qst):
                items = []
                for hp in range(NPAIR):
                    items.append(lambda hp=hp: q_chain(tcn, xt, qst, hp))
                for hp in range(NPAIR):
                    items.append(lambda hp=hp: k_chain(tcn, xt, hp))
                for ts in range(4):
                    items.append(lambda ts=ts: v_chain(tcn, xt, ts))
                return items

            def op_item(qg, mrg, qc, half):
                op = psM.tile([128, 512], F32, tag="mm", name="op")
                for hp in range(NPAIR):
                    nc.tensor.matmul(
                        op[:],
                        mrg[:, hp, qc * 128 : (qc + 1) * 128],
                        wo_sb[:, hp, half * 512 : (half + 1) * 512],
                        start=(hp == 0),
                        stop=(hp == NPAIR - 1),
                    )
                yt = prb.tile([128, 512], BF16, tag="yt")
                nc.vector.tensor_copy(yt[:], op[:])
                nc.sync.dma_start(
                    y_d[
                        qg * QG + qc * 128 : qg * QG + (qc + 1) * 128,
                        half * 512 : (half + 1) * 512,
                    ],
                    yt[:],
                )

            def outproj_items(qg, mrg):
                return [
                    lambda qc=qc, half=half: op_item(qg, mrg, qc, half)
                    for qc in range(4)
                    for half in range(2)
                ]

            fill = []  # deferred PE work (proj chains / outproj) to slot
            # into attention's spare cycles, keeping the Act engine fed

            def normalize_hp(qg, hp, av_all, sum_h, mrg):
                """Reciprocal + broadcast + multiply for one head pair."""
                rcp = prb.tile([2, QG], F32, tag="rcp_h")
                nc.vector.reciprocal(rcp[:], sum_h[:])
                for j in range(2):
                    idx = 2 * hp + j
                    rs0 = prb.tile([1, QG], F32, tag="rs0")
                    nc.sync.dma_start(rs0[:], rcp[j : j + 1, :])
                    rb = prb.tile([DH, QG], F32, tag="rb", bufs=8)
                    nc.gpsimd.partition_broadcast(rb[:], rs0[:])
                    if j == 0:
                        nc.vector.tensor_tensor(
                            mrg[0:DH, hp, :], av_all[0:DH, idx, :], rb[:], MULT
                        )
                    else:
                        odt = prb.tile([DH, QG], BF16, tag="odt")
                        nc.vector.tensor_tensor(
                            odt[:], av_all[0:DH, idx, :], rb[:], MULT
                        )
                        nc.sync.dma_start(mrg[DH:128, hp, :], odt[:])

            def attention(qg, qst):
                """Causal attention + softmax normalize for query group qg.

                Drains the fill queue (next T-chunk projections, previous
                group's out-projection) into the PE's spare per-kt cycles.
                """
                ktmax = 4 * (qg + 1)
                n_kt = NPAIR * ktmax
                av_all = pnrm.tile([DH + 1, 2 * NPAIR, QG], BF16, tag="avsb")
                sum_sb = pnrm.tile([2 * NPAIR, QG], BF16, tag="sums")
                mrg = pnrm.tile([128, NPAIR, QG], BF16, tag="mrg")
                # spread fill items over the group's kt iterations, starting
                # a couple of kts in (lets input DMAs land first)
                n_items = len(fill)
                stride = max(1, (n_kt - 4) // max(1, n_items))
                kt_count = 0

                for hp in range(NPAIR):
                    av = [
                        psAV.tile([DH + 1, QG], F32, tag="av", name=f"av{j}")
                        for j in range(2)
                    ]

                    def emit_av(e_t, kt):
                        diag = kt - 4 * qg
                        dlt = 128 * diag if diag >= 0 else 0
                        for j in range(2):
                            nc.tensor.matmul(
                                av[j][:, dlt:],
                                v_sb[:, kt, 2 * hp + j, :],
                                e_t[:, j, dlt:],
                                start=(kt == 0),
                                stop=(kt == ktmax - 1),
                            )

                    # AV(kt-1) is emitted after exp(kt) so scores(kt+1) can
                    # run on the PE while exp(kt) occupies the Act engine —
                    # otherwise the exp chain starves every other iteration.
                    pend = None
                    for kt in range(ktmax):
                        diag = kt - 4 * qg
                        dlt = 128 * diag if diag >= 0 else 0
                        s_t = psS.tile([128, 2, QG], F32, tag="st")
                        for j in range(2):
                            nc.tensor.matmul(
                                s_t[:, j, dlt:],
                                kt_sb[
                                    64 * j : 64 * (j + 1),
                                    hp,
                                    kt * 128 : (kt + 1) * 128,
                                ],
                                qst[64 * j : 64 * (j + 1), hp, dlt:],
                                start=True,
                                stop=True,
                                tile_position=(64 * j, 0),
                            )
                        e_t = pet.tile([128, 2, QG], BF16, tag="exps")
                        nc.scalar.activation(
                            e_t[:, :, dlt:], s_t[:, :, dlt:], EXP, scale=SCALE
                        )
                        if diag >= 0:
                            for j in range(2):
                                nc.vector.tensor_tensor(
                                    e_t[:, j, dlt : dlt + 128],
                                    e_t[:, j, dlt : dlt + 128],
                                    tri_sb[:],
                                    MULT,
                                )
                        if pend is not None:
                            emit_av(*pend)
                        pend = (e_t, kt)
                        kt_count += 1
                        if (
                            fill
                            and kt_count >= 4
                            and (kt_count - 4) % stride == 0
                        ):
                            fill.pop(0)()
                    emit_av(*pend)
                    # drain PSUM to SBUF so the next head pair can accumulate
                    last = qg == NQG - 1
                    sum_h = (
                        prb.tile([2, QG], BF16, tag="sum_h", name="sum_h")
                        if last
                        else None
                    )
                    for j in range(2):
                        idx = 2 * hp + j
                        nc.vector.tensor_copy(av_all[:, idx, :], av[j][:])
                        dst = (
                            sum_h[j : j + 1, :]
                            if last
                            else sum_sb[idx : idx + 1, :]
                        )
                        nc.sync.dma_start(dst, av_all[DH : DH + 1, idx, :])
                    if last:
                        # last group: normalize per head pair immediately so
                        # the final out-projection isn't serialized at the end
                        normalize_hp(qg, hp, av_all, sum_h, mrg)

                if qg < NQG - 1:
                    # batched reciprocal of all 8 softmax-sum rows, then
                    # broadcasts (GpSimd) overlapping multiplies (DVE)
                    rcp = pnrm.tile([2 * NPAIR, QG], F32, tag="rcp")
                    nc.vector.reciprocal(rcp[:], sum_sb[:])
                    rbs = []
                    for idx in range(2 * NPAIR):
                        rs0 = prb.tile([1, QG], F32, tag="rs0")
                        nc.sync.dma_start(rs0[:], rcp[idx : idx + 1, :])
                        rb = prb.tile([DH, QG], F32, tag="rb", bufs=8)
                        nc.gpsimd.partition_broadcast(rb[:], rs0[:])
                        rbs.append(rb)
                    for hp in range(NPAIR):
                        for j in range(2):
                            idx = 2 * hp + j
                            if j == 0:
                                nc.vector.tensor_tensor(
                                    mrg[0:DH, hp, :], av_all[0:DH, idx, :],
                                    rbs[idx][:], MULT,
                                )
                            else:
                                odt = prb.tile([DH, QG], BF16, tag="odt")
                                nc.vector.tensor_tensor(
                                    odt[:], av_all[0:DH, idx, :], rbs[idx][:],
                                    MULT,
                                )
                                nc.sync.dma_start(mrg[DH:128, hp, :], odt[:])
                return mrg

            # Software pipeline: attention(t) drains a fill queue holding
            # proj(t+1) chains and outproj(t-1) pieces, so the PE's spare
            # per-kt cycles absorb them and the Act engine never waits on a
            # dedicated projection phase.
            qst = pq.tile([128, NPAIR, QG], BF16, tag="qst", name="qst")
            for it in proj_items(0, xt0, qst):
                it()
            xt_next = load_xt(1)
            qst_next = pq.tile([128, NPAIR, QG], BF16, tag="qst", name="qst")
            fill.extend(proj_items(1, xt_next, qst_next))
            for tcn in range(NQG):
                mrg = attention(tcn, qst)
                # anything not drained runs here (small query groups)
                while fill:
                    fill.pop(0)()
                qst = qst_next
                if tcn + 2 < NQG:
                    xt_next = load_xt(tcn + 2)
                    qst_next = pq.tile(
                        [128, NPAIR, QG], BF16, tag="qst", name="qst"
                    )
                    fill.extend(proj_items(tcn + 2, xt_next, qst_next))
                fill.extend(outproj_items(tcn, mrg))
            while fill:
                fill.pop(0)()
    nc.compile()
    return nc


_NC_CACHE = None


def _get_nc():
    global _NC_CACHE
    if _NC_CACHE is None:
        _NC_CACHE = build_nc()
    return _NC_CACHE


def make_in_maps(x, Wq, Wk, Wv, Wo):
    x = np.asarray(x, dtype=np.float32)
    Wq = np.asarray(Wq, dtype=np.float32)
    Wk = np.asarray(Wk, dtype=np.float32)
    Wv = np.asarray(Wv, dtype=np.float32)
    Wo = np.asarray(Wo, dtype=np.float32)
    tri = np.triu(np.ones((128, 128), dtype=np.float32)).astype(BF)
    in_maps = []
    for c in range(8):
        b, g = divmod(c, 2)
        rows = slice(HG * g, HG * (g + 1))
        in_maps.append(
            {
                "xT": np.ascontiguousarray(x[b].T).astype(BF),
                "wq": np.ascontiguousarray(Wq[rows].T).astype(BF),
                "wk": np.ascontiguousarray(Wk[rows].T).astype(BF),
                "wv": np.ascontiguousarray(Wv[rows].T).astype(BF),
                "wo": np.ascontiguousarray(Wo[:, rows].T).astype(BF),
                "tri": tri,
            }
        )
    return in_maps


def run(x, Wq, Wk, Wv, Wo, trace=False, **spmd_kwargs):
    nc = _get_nc()
    in_maps = make_in_maps(x, Wq, Wk, Wv, Wo)
    res = run_bass_kernel_spmd(
        nc, in_maps, core_ids=list(range(8)), trace=trace, **spmd_kwargs
    )
    parts = [np.asarray(r["y"]).astype(np.float32) for r in res.results]
    y = np.stack([parts[2 * b] + parts[2 * b + 1] for b in range(B)])
    return y, res


def kernel(x, Wq, Wk, Wv, Wo):
    y, _ = run(x, Wq, Wk, Wv, Wo, trace=False)
    return y

